# revision 1
# baseline (speedup 1.0000x reference)
"""Trainium2 Bass kernel for nn_ContrastLoss (contrastive PSD loss).

Math notes (validated against the jax reference):
  * The band (rfft bins 92..568 of a 4096-point DFT) excludes DC, so the
    mean subtraction in the reference is a no-op for the band PSD.
  * diag(D) == 0 for the pairwise-MSE matrix, and every _compare() term
    reduces to rank-1 statistics of the normalized PSD matrices:
        sum_ij D_ij * F = M*SSQ_a + N*SSQ_b - 2 * cs_a . cs_b
    with SSQ = sum of squared entries and cs = column sums.  So the NxN
    Gram matrix is never materialized; the device only produces per-core
    column sums and per-row (sum, sum-of-squares) statistics.
  * Even/odd frequency split: for even k, X_k = DFT_2048(x0+x1)[k]; for
    odd k, X_k = DFT_2048(x0-x1)[k] (x0/x1 = crop halves).  This halves
    both the matmul FLOPs and the DFT-matrix footprint.

Device work per core (1024 crops of the 8192 total):
  crops_T [blk][contract 128][e|d][chunk][crop 128]  (fp32r)
  x  W_e [2048, 478 = cos|sin even bins], W_d [2048, 476]   (fp32r)
  -> PSUM [128 crops, 478/476], ACT Square (+row-sum accum),
  -> DVE adds -> band PSD p [128, 477], row sum/sumsq,
  -> PE colsum matmul with lhsT = 1/rowsum  -> cs [1, 477].
Host combines the 8 cores' (cs, rowstats) in float64.
"""

import numpy as np

# Problem constants (hardcoded; kernel.py must be self-contained)
B, C, T = 2, 64, 32768
L = 4096
K_CROPS = 32
HALF = L // 2                  # 2048
N_ROWS = C * K_CROPS           # 2048 rows per PSD matrix
N_CORES = 8
ROWS_PER_CORE = N_ROWS * 4 // N_CORES   # 1024
NB = ROWS_PER_CORE // 128      # 8 row blocks per core
NCH = HALF // 128              # 16 contract chunks per half
K_EVEN = np.arange(92, 569, 2)  # 239 even band bins
K_ODD = np.arange(93, 568, 2)   # 238 odd band bins
FE = len(K_EVEN)               # 239
FO = len(K_ODD)                # 238
F = FE + FO                    # 477

_NC = None
_W_CACHE = None


def _band_tables():
    """Signed 12-bit DFT phase tables m_e, m_d (int16), grouped layout.

    Device computes W = sin(2*pi*m/4096); phase shifts bake in cos
    (+1024) and -sin (+2048).  m is centered to [-2048, 2048) to stay
    inside the Sin LUT domain [-pi, pi].
    """
    global _W_CACHE
    if _W_CACHE is not None:
        return _W_CACHE
    NG = 8
    n = np.arange(HALF, dtype=np.int64)[:, None]

    def mk(ks, fbins):
        nk = n * ks[None, :]
        m = np.concatenate([nk + 1024, nk + 2048], axis=1)   # cos | -sin
        m = ((m + 2048) % 4096) - 2048
        m = m.astype(np.int16).reshape(NG, NCH // NG, 128, 2 * fbins)
        return np.ascontiguousarray(m.transpose(0, 2, 1, 3))

    _W_CACHE = (mk(K_EVEN, FE), mk(K_ODD, FO))
    return _W_CACHE


def _build_module():
    global _NC
    if _NC is not None:
        return _NC
    import concourse.bacc as bacc
    import concourse.bass as bass
    import concourse.tile as tile
    from concourse import mybir

    f32 = mybir.dt.float32
    f32r = mybir.dt.float32r
    AF = mybir.ActivationFunctionType
    ALU = mybir.AluOpType

    import math
    i16 = mybir.dt.int16

    nc = bacc.Bacc("TRN2", target_bir_lowering=False, debug=False,
                   num_devices=N_CORES)
    # chunk-outer passes over row blocks: {0,1,2}, {3,4,5}, {6,7}
    passes = [[0, 1, 2], [3, 4, 5], [6, 7]]
    NG = 8                       # chunk groups of 2 for DMA batching
    # crops: [half, chunk-group, partition, chunk-in-group, crop-col]
    crops_d = [
        nc.dram_tensor(f"crops_p{p}", [2, NG, 128, NCH // NG,
                                       128 * len(blks)], f32r,
                       kind="ExternalInput")
        for p, blks in enumerate(passes)
    ]
    # signed 12-bit DFT phases; device computes W = sin(2*pi*m/4096)
    m_e = nc.dram_tensor("m_e", [NG, 128, NCH // NG, 2 * FE], i16,
                         kind="ExternalInput")
    m_d = nc.dram_tensor("m_d", [NG, 128, NCH // NG, 2 * FO], i16,
                         kind="ExternalInput")
    CPG = NCH // NG              # chunks per group (2)
    out_cs = nc.dram_tensor("out_cs", [1, F], f32, kind="ExternalOutput")
    FP = F + 1   # fp32r matmul needs an even moving free dim; pad with zeros
    out_rq = nc.dram_tensor("out_rq", [128, 2 * NB], f32,
                            kind="ExternalOutput")
    SIN_SCALE = 2.0 * math.pi / 4096.0

    with tile.TileContext(nc) as tc:
        with (
            tc.tile_pool(name="wp", bufs=1) as wp,
            tc.tile_pool(name="mp", bufs=3) as mp,
            tc.tile_pool(name="cp", bufs=2) as cp,
            tc.tile_pool(name="sq", bufs=3) as sqp,
            tc.tile_pool(name="pp", bufs=3) as ppool,
            tc.tile_pool(name="sm", bufs=6) as sm,
            tc.tile_pool(name="outp", bufs=1) as outp,
            tc.tile_pool(name="ps", bufs=7, space=bass.MemorySpace.PSUM) as ps,
            tc.tile_pool(name="pcs", bufs=1, space=bass.MemorySpace.PSUM) as pcs,
        ):
            we_t = wp.tile([128, NCH, 2 * FE], f32r)
            wd_t = wp.tile([128, NCH, 2 * FO], f32r)
            rq_t = outp.tile([128, 2 * NB], f32)
            zero_col = outp.tile([128, 1], f32)
            nc.vector.memset(zero_col, 0.0)
            cs_psum = pcs.tile([1, FP], f32)

            pending = []   # (inv, p_t, blk) colsum matmuls deferred one pass

            for p, blks in enumerate(passes):
                nbp = len(blks)
                cpass = cp.tile([128, 2, NCH, 128 * nbp], f32r, tag="cp")
                # DMA in consumption order; W-phase load+gen during pass 0
                for g in range(NG):
                    if p == 0:
                        me_g = mp.tile([128, CPG, 2 * FE], i16,
                                       tag="me")
                        md_g = mp.tile([128, CPG, 2 * FO], i16,
                                       tag="md")
                        nc.sync.dma_start(out=me_g, in_=m_e[g])
                        nc.sync.dma_start(out=md_g, in_=m_d[g])
                    nc.sync.dma_start(
                        out=cpass[:, 0, CPG * g:CPG * (g + 1), :],
                        in_=crops_d[p][0, g])
                    nc.sync.dma_start(
                        out=cpass[:, 1, CPG * g:CPG * (g + 1), :],
                        in_=crops_d[p][1, g])
                    if p == 0:
                        for ci in range(CPG):
                            ch = CPG * g + ci
                            cve = ppool.tile([128, 2 * FE], f32, tag="cve")
                            cvd = ppool.tile([128, 2 * FO], f32, tag="cvd")
                            nc.vector.tensor_copy(cve, me_g[:, ci, :])
                            nc.vector.tensor_copy(cvd, md_g[:, ci, :])
                            with nc.allow_low_precision(
                                    reason="fp32r same width as fp32"):
                                nc.scalar.activation(
                                    out=we_t[:, ch, :], in_=cve,
                                    func=AF.Sin, scale=SIN_SCALE)
                                nc.scalar.activation(
                                    out=wd_t[:, ch, :], in_=cvd,
                                    func=AF.Sin, scale=SIN_SCALE)

                pe_ts = [ps.tile([128, 2 * FE], f32, tag="ps",
                                 name=f"pe{p}_{j}") for j in range(nbp)]
                pd_ts = [ps.tile([128, 2 * FO], f32, tag="ps",
                                 name=f"pd{p}_{j}") for j in range(nbp)]

                def emit_cs(items):
                    for c_inv, c_p, c_blk in items:
                        nc.tensor.matmul(cs_psum, c_inv, c_p,
                                         start=(c_blk == 0),
                                         stop=(c_blk == NB - 1))

                def emit_post(j, blk):
                    sq_e = sqp.tile([128, 2 * FE], f32, tag="sqe",
                                    name=f"sqe{blk}")
                    sq_d = sqp.tile([128, 2 * FO], f32, tag="sqd",
                                    name=f"sqd{blk}")
                    acc_e = sm.tile([128, 1], f32, tag="acce",
                                    name=f"acce{blk}")
                    acc_d = sm.tile([128, 1], f32, tag="accd",
                                    name=f"accd{blk}")
                    nc.scalar.activation(out=sq_e, in_=pe_ts[j],
                                         func=AF.Square, accum_out=acc_e)
                    nc.scalar.activation(out=sq_d, in_=pd_ts[j],
                                         func=AF.Square, accum_out=acc_d)
                    p_t = ppool.tile([128, FP], f32r, tag="p",
                                     name=f"p{blk}")
                    with nc.allow_low_precision(reason="fp32r is fp32-width"):
                        nc.vector.tensor_add(p_t[:, 0:FE], sq_e[:, 0:FE],
                                             sq_e[:, FE:2 * FE])
                        nc.vector.tensor_add(p_t[:, FE:F], sq_d[:, 0:FO],
                                             sq_d[:, FO:2 * FO])
                        nc.vector.tensor_copy(p_t[:, F:FP], zero_col)
                    rs = rq_t[:, 2 * blk:2 * blk + 1]
                    nc.vector.tensor_add(rs, acc_e, acc_d)
                    psq = ppool.tile([128, F], f32, tag="psq",
                                     name=f"psq{blk}")
                    nc.scalar.activation(
                        out=psq, in_=p_t[:, 0:F], func=AF.Square,
                        accum_out=rq_t[:, 2 * blk + 1:2 * blk + 2])
                    inv = sm.tile([128, 1], f32r, tag="inv",
                                  name=f"inv{blk}")
                    with nc.allow_low_precision(reason="fp32r is fp32-width"):
                        nc.vector.reciprocal(inv, rs)
                    pending.append((inv, p_t, blk))

                if p < len(passes) - 1:
                    # chunk-outer: follow DMA arrival order
                    for ch in range(NCH):
                        se = (ch == 0)
                        sp = (ch == NCH - 1)
                        for j in range(nbp):
                            nc.tensor.matmul(
                                pe_ts[j],
                                cpass[:, 0, ch, 128 * j:128 * (j + 1)],
                                we_t[:, ch, :], start=se, stop=sp)
                        for j in range(nbp):
                            nc.tensor.matmul(
                                pd_ts[j],
                                cpass[:, 1, ch, 128 * j:128 * (j + 1)],
                                wd_t[:, ch, :], start=se, stop=sp)
                        if ch == 4 and pending:
                            emit_cs(pending)
                            pending = []
                    for j, blk in enumerate(blks):
                        emit_post(j, blk)
                else:
                    # final pass: block-outer so earlier blocks' post-chains
                    # overlap later blocks' matmuls (shorter kernel tail)
                    for j, blk in enumerate(blks):
                        for ch in range(NCH):
                            nc.tensor.matmul(
                                pe_ts[j],
                                cpass[:, 0, ch, 128 * j:128 * (j + 1)],
                                we_t[:, ch, :], start=(ch == 0),
                                stop=(ch == NCH - 1))
                        for ch in range(NCH):
                            nc.tensor.matmul(
                                pd_ts[j],
                                cpass[:, 1, ch, 128 * j:128 * (j + 1)],
                                wd_t[:, ch, :], start=(ch == 0),
                                stop=(ch == NCH - 1))
                        if j == 0 and pending:
                            emit_cs(pending)
                            pending = []
                        if j > 0:
                            emit_cs(pending)
                            pending = []
                        emit_post(j, blk)

            emit_cs(pending)

            cs_sb = outp.tile([1, F], f32)
            nc.vector.tensor_copy(cs_sb, cs_psum[:, 0:F])
            nc.sync.dma_start(out=out_cs[:], in_=cs_sb)
            nc.sync.dma_start(out=out_rq[:], in_=rq_t)

    nc.compile()
    _NC = nc
    return nc


def _gather_crops(sig, offs_flat):
    """sig [T] -> crops [len(offs), L] float32."""
    from numpy.lib.stride_tricks import sliding_window_view
    win = sliding_window_view(sig, L)
    return win[offs_flat].astype(np.float32, copy=False)


_PASSES = [[0, 1, 2], [3, 4, 5], [6, 7]]


def _core_input(rows_ed):
    """rows_ed: (e, d) each [1024, 2048] f32 -> per-pass crop tensors.

    Layout [half, chunk-group, partition, chunk-in-group, crop-col] so
    each DMA moves 4-chunk-wide contiguous per-partition lines.
    """
    e, d = rows_ed
    NG = 8
    # [row, n] -> [grp, partition, chunk-in-group, row]
    eR = e.reshape(ROWS_PER_CORE, NG, NCH // NG, 128).transpose(1, 3, 2, 0)
    dR = d.reshape(ROWS_PER_CORE, NG, NCH // NG, 128).transpose(1, 3, 2, 0)
    out = {}
    for p, blks in enumerate(_PASSES):
        c0 = blks[0] * 128
        c1 = c0 + 128 * len(blks)
        out[f"crops_p{p}"] = np.ascontiguousarray(
            np.stack([eR[..., c0:c1], dR[..., c0:c1]], axis=0),
            dtype=np.float32)
    return out


def _host_prepare(model_output, GT_sig, offsets_st, offsets_t):
    """Build per-core in_maps."""
    m_e, m_d = _band_tables()
    in_maps = []
    mats = []   # 4 matrices' (e, d) row data [2048, 2048] each
    for b in range(B):
        offs = np.asarray(offsets_st[b], dtype=np.int64).reshape(-1)
        ch_idx = np.repeat(np.arange(C), K_CROPS)
        base = np.asarray(model_output[b], dtype=np.float32)
        from numpy.lib.stride_tricks import sliding_window_view
        win = sliding_window_view(base, L, axis=-1)  # [C, T-L+1, L]
        cr = win[ch_idx, offs]                       # [2048, L]
        mats.append((cr[:, :HALF] + cr[:, HALF:],
                     cr[:, :HALF] - cr[:, HALF:]))
    for b in range(B):
        offs = np.asarray(offsets_t[b], dtype=np.int64).reshape(-1)
        cr = _gather_crops(np.asarray(GT_sig[b], dtype=np.float32), offs)
        mats.append((cr[:, :HALF] + cr[:, HALF:],
                     cr[:, :HALF] - cr[:, HALF:]))
    for m in range(4):
        e, d = mats[m]
        for h in range(2):
            sl = slice(h * ROWS_PER_CORE, (h + 1) * ROWS_PER_CORE)
            im = {"m_e": m_e, "m_d": m_d}
            im.update(_core_input((e[sl], d[sl])))
            in_maps.append(im)
    return in_maps


def _combine(results, label_flag):
    """results: list of 8 dicts with out_cs [1,F], out_rq [128,2*NB]."""
    cs = np.zeros((4, F), dtype=np.float64)
    ssq = np.zeros(4, dtype=np.float64)
    for m in range(4):
        for h in range(2):
            r = results[2 * m + h]
            cs[m] += np.asarray(r["out_cs"], dtype=np.float64)[0]
            rq = np.asarray(r["out_rq"], dtype=np.float64)
            rs = rq[:, 0::2]
            q = rq[:, 1::2]
            ssq[m] += float(np.sum(q / (rs * rs)))

    N = float(N_ROWS)

    def cmp_excl(a):
        return (2.0 * N * ssq[a] - 2.0 * np.dot(cs[a], cs[a])) / F / (N * (N - 1.0))

    def cmp_full(a, b):
        return (N * ssq[a] + N * ssq[b] - 2.0 * np.dot(cs[a], cs[b])) / F / (N * N)

    lf = np.asarray(label_flag, dtype=np.float64).reshape(-1)
    lf_sum = lf[0] + lf[1]
    denom = 1.0 if lf_sum == 0 else lf_sum
    pos_loss = (cmp_excl(0) + cmp_excl(1)) / 2.0
    neg_loss = -cmp_full(0, 1)
    pos_GT = (lf[0] * cmp_full(0, 2) + lf[1] * cmp_full(1, 3)) / denom
    neg_GT = -(lf[0] * cmp_full(1, 2) + lf[1] * cmp_full(0, 3)) / denom
    if lf_sum == 0:
        pos_GT = 0.0
        neg_GT = 0.0
    loss = pos_loss + neg_loss + pos_GT + neg_GT
    return (np.float32(loss), np.float32(pos_loss), np.float32(neg_loss),
            np.float32(pos_GT), np.float32(neg_GT))


def run(inputs, trace=False):
    """Returns (outputs_tuple, BassKernelResults)."""
    from concourse import bass_utils
    nc = _build_module()
    in_maps = _host_prepare(
        inputs["model_output"], inputs["GT_sig"],
        inputs["offsets_st"], inputs["offsets_t"])
    res = bass_utils.run_bass_kernel_spmd(
        nc, in_maps, core_ids=list(range(N_CORES)), trace=trace)
    outs = _combine(res.results, inputs["label_flag"])
    return outs, res


def kernel(**inputs):
    outs, _ = run(inputs)
    return outs



# revision 3
# speedup vs baseline: 1.5393x; 1.5393x over previous
"""Trainium2 Bass kernel for nn_ContrastLoss (contrastive PSD loss).

Math notes (validated against the jax reference):
  * The band (rfft bins 92..568 of a 4096-point DFT) excludes DC, so the
    mean subtraction in the reference is a no-op for the band PSD.
  * diag(D) == 0 for the pairwise-MSE matrix, and every _compare() term
    reduces to rank-1 statistics of the normalized PSD matrices:
        sum_ij D_ij * F = M*SSQ_a + N*SSQ_b - 2 * cs_a . cs_b
    with SSQ = sum of squared entries and cs = column sums.  So the NxN
    Gram matrix is never materialized; the device only produces per-core
    column sums and per-row (sum, sum-of-squares) statistics.
  * Even/odd frequency split: for even k, X_k = DFT_2048(x0+x1)[k]; for
    odd k, X_k = DFT_2048(x0-x1)[k] (x0/x1 = crop halves).  This halves
    both the matmul FLOPs and the DFT-matrix footprint.
  * All matmul operands are fp8-e4m3 with DoubleRow perf mode (two
    128-deep k-tiles per instruction).  An end-to-end numpy simulation
    of e4m3 quantization gives rel err ~7e-4 on the loss terms (the
    quantization bias is common-mode across the four PSD matrices and
    cancels in the signed loss sum), far under the 2e-2 gate.

Device work per core (1024 crops of the 8192 total):
  DMA in: W_e [128,16,478], W_d [128,16,476] fp8 (~1.9 MB) then 8 crop
  blocks [128, 2(e|d), 16, 128] fp8 (0.5 MB each).  Per block: 32
  DoubleRow matmuls -> PSUM [128,478]/[128,476], ACT Square (+row-sum
  accum), DVE adds -> band PSD p [128,477], row sum/sumsq, PE colsum
  matmul with lhsT = 1/rowsum -> cs [1,477].  Host combines the 8
  cores' (cs, rowstats) in float64.
"""

import numpy as np

# Problem constants (hardcoded; kernel.py must be self-contained)
B, C, T = 2, 64, 32768
L = 4096
K_CROPS = 32
HALF = L // 2                  # 2048
N_ROWS = C * K_CROPS           # 2048 rows per PSD matrix
N_CORES = 8
ROWS_PER_CORE = N_ROWS * 4 // N_CORES   # 1024
NB = ROWS_PER_CORE // 128      # 8 row blocks per core
NCH = HALF // 128              # 16 contract chunks per half
NCP = NCH // 2                 # 8 chunk PAIRS (DoubleRow granularity)
K_EVEN = np.arange(92, 569, 2)  # 239 even band bins
K_ODD = np.arange(93, 568, 2)   # 238 odd band bins
FE = len(K_EVEN)               # 239
FO = len(K_ODD)                # 238
F = FE + FO                    # 477

_NC = None
_W_CACHE = None


def _band_mats():
    """fp8-e4m3 DFT matrices in device layout [128, NCH, 2*Fbins].

    Column layout per half: [cos | -sin] blocks (squared later, sign
    irrelevant).  Row n = 128*chunk + partition.
    """
    global _W_CACHE
    if _W_CACHE is not None:
        return _W_CACHE
    import ml_dtypes
    n = np.arange(HALF, dtype=np.float64)[:, None]

    def mk(ks):
        ang = 2.0 * np.pi * (n * ks[None, :]) / float(L)
        w = np.concatenate([np.cos(ang), -np.sin(ang)], axis=1)
        w8 = w.astype(np.float32).astype(ml_dtypes.float8_e4m3)
        return np.ascontiguousarray(
            w8.reshape(NCH, 128, 2 * len(ks)).transpose(1, 0, 2))

    _W_CACHE = (mk(K_EVEN), mk(K_ODD))
    return _W_CACHE


def _build_module():
    global _NC
    if _NC is not None:
        return _NC
    import concourse.bacc as bacc
    import concourse.bass as bass
    import concourse.tile as tile
    from concourse import mybir

    f32 = mybir.dt.float32
    f32r = mybir.dt.float32r
    f8 = mybir.dt.float8e4
    AF = mybir.ActivationFunctionType
    DR = mybir.MatmulPerfMode.DoubleRow

    nc = bacc.Bacc("TRN2", target_bir_lowering=False, debug=False,
                   num_devices=N_CORES)

    # crops: [block, e|d, partition, chunk, crop] fp8
    crops_d = nc.dram_tensor("crops", [NB, 2, 128, NCH, 128], f8,
                             kind="ExternalInput")
    w_e_d = nc.dram_tensor("w_e", [128, NCH, 2 * FE], f8,
                           kind="ExternalInput")
    w_d_d = nc.dram_tensor("w_d", [128, NCH, 2 * FO], f8,
                           kind="ExternalInput")
    out_cs = nc.dram_tensor("out_cs", [1, F], f32, kind="ExternalOutput")
    FP = F + 1   # fp32r matmul needs an even moving free dim; pad with zeros
    out_rq = nc.dram_tensor("out_rq", [128, 2 * NB], f32,
                            kind="ExternalOutput")

    with tile.TileContext(nc) as tc:
        with (
            tc.tile_pool(name="wp", bufs=1) as wp,
            tc.tile_pool(name="cp", bufs=NB) as cp,
            tc.tile_pool(name="sq", bufs=3) as sqp,
            tc.tile_pool(name="pp", bufs=3) as ppool,
            tc.tile_pool(name="sm", bufs=6) as sm,
            tc.tile_pool(name="outp", bufs=1) as outp,
            tc.tile_pool(name="ps", bufs=6, space=bass.MemorySpace.PSUM) as ps,
            tc.tile_pool(name="pcs", bufs=1, space=bass.MemorySpace.PSUM) as pcs,
        ):
            we_t = wp.tile([128, NCH, 2 * FE], f8)
            wd_t = wp.tile([128, NCH, 2 * FO], f8)
            rq_t = outp.tile([128, 2 * NB], f32)
            zero_col = outp.tile([128, 1], f32)
            nc.vector.memset(zero_col, 0.0)
            cs_psum = pcs.tile([1, FP], f32)

            # All DMAs up front, in consumption order; the DMA queue
            # streams while the PE chases it block by block.
            nc.sync.dma_start(out=we_t, in_=w_e_d[:])
            nc.sync.dma_start(out=wd_t, in_=w_d_d[:])
            cpb = []
            for blk in range(NB):
                ct = cp.tile([128, 2, NCH, 128], f8, tag="cp",
                             name=f"c{blk}")
                cpb.append(ct)
                nc.sync.dma_start(out=ct[:, 0], in_=crops_d[blk, 0])
                nc.sync.dma_start(out=ct[:, 1], in_=crops_d[blk, 1])

            for blk in range(NB):
                pe_t = ps.tile([128, 2 * FE], f32, tag="ps",
                               name=f"pe{blk}")
                pd_t = ps.tile([128, 2 * FO], f32, tag="ps",
                               name=f"pd{blk}")
                # DoubleRow: lhsT [128, 2, 128] (two k-tiles of crops),
                # rhs [128, 2, Ncols] (same two k-tiles of W).
                for half, w_t, p_t, fb in ((0, we_t, pe_t, FE),
                                           (1, wd_t, pd_t, FO)):
                    for cs_lo in (0, fb):
                        for c in range(NCP):
                            nc.tensor.matmul(
                                p_t[:, cs_lo:cs_lo + fb],
                                cpb[blk][:, half, 2 * c:2 * c + 2, :],
                                w_t[:, 2 * c:2 * c + 2, cs_lo:cs_lo + fb],
                                start=(c == 0), stop=(c == NCP - 1),
                                perf_mode=DR)

                # Post: PSD, row stats, normalized column sums.
                sq_e = sqp.tile([128, 2 * FE], f32, tag="sqe",
                                name=f"sqe{blk}")
                sq_d = sqp.tile([128, 2 * FO], f32, tag="sqd",
                                name=f"sqd{blk}")
                acc_e = sm.tile([128, 1], f32, tag="acce", name=f"ae{blk}")
                acc_d = sm.tile([128, 1], f32, tag="accd", name=f"ad{blk}")
                nc.scalar.activation(out=sq_e, in_=pe_t, func=AF.Square,
                                     accum_out=acc_e)
                nc.scalar.activation(out=sq_d, in_=pd_t, func=AF.Square,
                                     accum_out=acc_d)
                p_t = ppool.tile([128, FP], f32r, tag="p", name=f"p{blk}")
                with nc.allow_low_precision(reason="fp32r is fp32-width"):
                    nc.vector.tensor_add(p_t[:, 0:FE], sq_e[:, 0:FE],
                                         sq_e[:, FE:2 * FE])
                    nc.vector.tensor_add(p_t[:, FE:F], sq_d[:, 0:FO],
                                         sq_d[:, FO:2 * FO])
                    nc.vector.tensor_copy(p_t[:, F:FP], zero_col)
                rs = rq_t[:, 2 * blk:2 * blk + 1]
                nc.vector.tensor_add(rs, acc_e, acc_d)
                psq = ppool.tile([128, F], f32, tag="psq", name=f"q{blk}")
                nc.scalar.activation(
                    out=psq, in_=p_t[:, 0:F], func=AF.Square,
                    accum_out=rq_t[:, 2 * blk + 1:2 * blk + 2])
                inv = sm.tile([128, 1], f32r, tag="inv", name=f"i{blk}")
                with nc.allow_low_precision(reason="fp32r is fp32-width"):
                    nc.vector.reciprocal(inv, rs)
                nc.tensor.matmul(cs_psum, inv, p_t,
                                 start=(blk == 0), stop=(blk == NB - 1))

            cs_sb = outp.tile([1, F], f32)
            nc.vector.tensor_copy(cs_sb, cs_psum[:, 0:F])
            nc.sync.dma_start(out=out_cs[:], in_=cs_sb)
            nc.sync.dma_start(out=out_rq[:], in_=rq_t)

    nc.compile()
    _NC = nc
    return nc


def _core_input(rows_ed):
    """rows_ed: (e, d) each [1024, 2048] f32 -> crops [NB,2,128,NCH,128] fp8."""
    import ml_dtypes
    e, d = rows_ed
    out = np.empty((NB, 2, 128, NCH, 128), dtype=ml_dtypes.float8_e4m3)
    for h, m in ((0, e), (1, d)):
        # [blk, crop, chunk, part] -> [blk, part, chunk, crop]
        q = m.astype(ml_dtypes.float8_e4m3)
        out[:, h] = q.reshape(NB, 128, NCH, 128).transpose(0, 3, 2, 1)
    return {"crops": out}


def _host_prepare(model_output, GT_sig, offsets_st, offsets_t):
    """Build per-core in_maps."""
    w_e, w_d = _band_mats()
    from numpy.lib.stride_tricks import sliding_window_view
    in_maps = []
    mats = []   # 4 matrices' (e, d) row data [2048, 2048] each
    for b in range(B):
        offs = np.asarray(offsets_st[b], dtype=np.int64).reshape(-1)
        ch_idx = np.repeat(np.arange(C), K_CROPS)
        base = np.asarray(model_output[b], dtype=np.float32)
        win = sliding_window_view(base, L, axis=-1)  # [C, T-L+1, L]
        cr = win[ch_idx, offs]                       # [2048, L]
        mats.append((cr[:, :HALF] + cr[:, HALF:],
                     cr[:, :HALF] - cr[:, HALF:]))
    for b in range(B):
        offs = np.asarray(offsets_t[b], dtype=np.int64).reshape(-1)
        win = sliding_window_view(
            np.asarray(GT_sig[b], dtype=np.float32), L)
        cr = win[offs]
        mats.append((cr[:, :HALF] + cr[:, HALF:],
                     cr[:, :HALF] - cr[:, HALF:]))
    for m in range(4):
        e, d = mats[m]
        for h in range(2):
            sl = slice(h * ROWS_PER_CORE, (h + 1) * ROWS_PER_CORE)
            im = {"w_e": w_e, "w_d": w_d}
            im.update(_core_input((e[sl], d[sl])))
            in_maps.append(im)
    return in_maps


def _combine(results, label_flag):
    """results: list of 8 dicts with out_cs [1,F], out_rq [128,2*NB]."""
    cs = np.zeros((4, F), dtype=np.float64)
    ssq = np.zeros(4, dtype=np.float64)
    for m in range(4):
        for h in range(2):
            r = results[2 * m + h]
            cs[m] += np.asarray(r["out_cs"], dtype=np.float64)[0]
            rq = np.asarray(r["out_rq"], dtype=np.float64)
            rs = rq[:, 0::2]
            q = rq[:, 1::2]
            ssq[m] += float(np.sum(q / (rs * rs)))

    N = float(N_ROWS)

    def cmp_excl(a):
        return (2.0 * N * ssq[a] - 2.0 * np.dot(cs[a], cs[a])) / F / (N * (N - 1.0))

    def cmp_full(a, b):
        return (N * ssq[a] + N * ssq[b] - 2.0 * np.dot(cs[a], cs[b])) / F / (N * N)

    lf = np.asarray(label_flag, dtype=np.float64).reshape(-1)
    lf_sum = lf[0] + lf[1]
    denom = 1.0 if lf_sum == 0 else lf_sum
    pos_loss = (cmp_excl(0) + cmp_excl(1)) / 2.0
    neg_loss = -cmp_full(0, 1)
    pos_GT = (lf[0] * cmp_full(0, 2) + lf[1] * cmp_full(1, 3)) / denom
    neg_GT = -(lf[0] * cmp_full(1, 2) + lf[1] * cmp_full(0, 3)) / denom
    if lf_sum == 0:
        pos_GT = 0.0
        neg_GT = 0.0
    loss = pos_loss + neg_loss + pos_GT + neg_GT
    return (np.float32(loss), np.float32(pos_loss), np.float32(neg_loss),
            np.float32(pos_GT), np.float32(neg_GT))


def run(inputs, trace=False):
    """Returns (outputs_tuple, BassKernelResults)."""
    from concourse import bass_utils
    nc = _build_module()
    in_maps = _host_prepare(
        inputs["model_output"], inputs["GT_sig"],
        inputs["offsets_st"], inputs["offsets_t"])
    res = bass_utils.run_bass_kernel_spmd(
        nc, in_maps, core_ids=list(range(N_CORES)), trace=trace)
    outs = _combine(res.results, inputs["label_flag"])
    return outs, res


def kernel(**inputs):
    outs, _ = run(inputs)
    return outs


# revision 7
# speedup vs baseline: 2.1469x; 1.3947x over previous
"""Trainium2 Bass kernel for nn_ContrastLoss (contrastive PSD loss).

Math notes (validated against the jax reference):
  * The band (rfft bins 92..568 of a 4096-point DFT) excludes DC, so the
    mean subtraction in the reference is a no-op for the band PSD.
  * diag(D) == 0 for the pairwise-MSE matrix, and every _compare() term
    reduces to rank-1 statistics of the normalized PSD matrices:
        sum_ij D_ij * F = M*SSQ_a + N*SSQ_b - 2 * cs_a . cs_b
    with SSQ = sum of squared entries and cs = column sums.  So the NxN
    Gram matrix is never materialized; the device only produces per-core
    column sums and per-row (sum, sum-of-squares) statistics.
  * Radix-2 DIF splits, applied where the sub-signals stay real:
      k odd        : X_k = DFT(d)[k],   d  = x0 - x1          [2048]
      k = 2 mod 4  : X_k = DFT(eo)[k],  eo = e0 - e1          [1024]
      k = 0 mod 4  : X_k = DFT(ee)[k],  ee = e0 + e1          [1024]
    with e = x0 + x1 and phases e^(-2*pi*i*m*k/4096) folded into the
    (real) DFT matrices.  This cuts matmul MACs to 75% of the single
    split and the DFT-matrix bytes to 1.46 MB.
  * All matmul operands are fp8-e4m3 with DoubleRow perf mode (two
    128-deep k-tiles per instruction).  An end-to-end numpy simulation
    of e4m3 quantization gives rel err ~7e-4 (the quantization bias is
    common-mode across the four PSD matrices and mostly cancels in the
    signed loss sum), far under the 2e-2 gate.

Device schedule per core (1024 crops of the 8192 total):
  Big-line DMAs (few descriptors, 3.8-8 KB per partition): W_even,
  crop-group 0, W_odd, crop-groups 1-3.  Dummy fp8 matmuls pre-ramp the
  PE p-state while DMA streams.  Per 128-crop block: 24 DoubleRow
  matmuls -> 3 PSUM tiles, ACT Square (+row-sum accum), DVE adds ->
  band PSD p [128,477], row sum/sumsq, PE colsum matmul with
  lhsT = 1/rowsum -> cs [1,477].  Host combines in float64.
"""

import numpy as np

# Problem constants (hardcoded; kernel.py must be self-contained)
B, C, T = 2, 64, 32768
L = 4096
K_CROPS = 32
HALF = L // 2                  # 2048
QUART = L // 4                 # 1024
N_ROWS = C * K_CROPS           # 2048 rows per PSD matrix
N_CORES = 8
ROWS_PER_CORE = N_ROWS * 4 // N_CORES   # 1024
NB = ROWS_PER_CORE // 128      # 8 row blocks per core
NGRP = 4                       # crop DMA groups (2 blocks each)
NSLOT = 32                     # 128-sample chunk slots per crop: 8 ee, 8 eo, 16 d
K_EE = np.arange(92, 569, 4)   # 120 bins, k = 0 mod 4
K_EO = np.arange(94, 567, 4)   # 119 bins, k = 2 mod 4
K_OD = np.arange(93, 568, 2)   # 238 bins, k odd
FEE, FEO, FOD = len(K_EE), len(K_EO), len(K_OD)
F = FEE + FEO + FOD            # 477
FP = F + 1                     # pad col so fp32r matmul free dim is even
N_WARM = 26                    # PE p-state pre-ramp matmuls

_NC = None
_W_CACHE = None


def _band_mats():
    """fp8-e4m3 DFT matrices in device layout [128, nch, cols].

    Returns (w_even [128, 8, 478], w_odd [128, 16, 476]).  w_even packs
    the ee columns [cos|_-sin] (240) then eo columns (238); row
    m = 128*chunk + partition.  Column layout per class: [cos | -sin].
    """
    global _W_CACHE
    if _W_CACHE is not None:
        return _W_CACHE
    import ml_dtypes

    def mk(ks, n):
        m = np.arange(n, dtype=np.float64)[:, None]
        ang = 2.0 * np.pi * (m * ks[None, :]) / float(L)
        w = np.concatenate([np.cos(ang), -np.sin(ang)], axis=1)
        return w.astype(np.float32).astype(ml_dtypes.float8_e4m3)

    w_ev = np.concatenate([mk(K_EE, QUART), mk(K_EO, QUART)], axis=1)
    w_ev = np.ascontiguousarray(
        w_ev.reshape(QUART // 128, 128, 2 * FEE + 2 * FEO).transpose(1, 0, 2))
    w_od = mk(K_OD, HALF)
    w_od = np.ascontiguousarray(
        w_od.reshape(HALF // 128, 128, 2 * FOD).transpose(1, 0, 2))
    _W_CACHE = (w_ev, w_od)
    return _W_CACHE


def _build_module():
    global _NC
    if _NC is not None:
        return _NC
    import concourse.bacc as bacc
    import concourse.bass as bass
    import concourse.tile as tile
    from concourse import mybir

    f32 = mybir.dt.float32
    f32r = mybir.dt.float32r
    f8 = mybir.dt.float8e4
    AF = mybir.ActivationFunctionType
    DR = mybir.MatmulPerfMode.DoubleRow

    CEV = 2 * FEE + 2 * FEO    # 478 w_even cols
    COD = 2 * FOD              # 476 w_odd cols

    nc = bacc.Bacc("TRN2", target_bir_lowering=False, debug=False,
                   num_devices=N_CORES)

    # crops: [grp, partition, blk-in-grp, slot, crop] fp8, 8 KB lines
    crops_d = nc.dram_tensor("crops", [NGRP, 128, 2, NSLOT, 128], f8,
                             kind="ExternalInput")
    w_ev_d = nc.dram_tensor("w_ev", [128, 8, CEV], f8, kind="ExternalInput")
    w_od_d = nc.dram_tensor("w_od", [128, 16, COD], f8, kind="ExternalInput")
    out_cs = nc.dram_tensor("out_cs", [1, F], f32, kind="ExternalOutput")
    out_rq = nc.dram_tensor("out_rq", [128, 2 * NB], f32,
                            kind="ExternalOutput")

    with tile.TileContext(nc) as tc:
        with (
            tc.tile_pool(name="wp", bufs=1) as wp,
            tc.tile_pool(name="cp", bufs=NGRP) as cp,
            tc.tile_pool(name="sq", bufs=4) as sqp,
            tc.tile_pool(name="pp", bufs=3) as ppool,
            tc.tile_pool(name="sm", bufs=8) as sm,
            tc.tile_pool(name="outp", bufs=1) as outp,
            tc.tile_pool(name="ps", bufs=6, space=bass.MemorySpace.PSUM) as ps,
            tc.tile_pool(name="pcs", bufs=1, space=bass.MemorySpace.PSUM) as pcs,
        ):
            we_t = wp.tile([128, 8, CEV], f8)
            wd_t = wp.tile([128, 16, COD], f8)
            rq_t = outp.tile([128, 2 * NB], f32)
            zero_col = outp.tile([128, 1], f32)
            warm_t = outp.tile([128, 2, 256], f8)
            nc.vector.memset(zero_col, 0.0)
            nc.vector.memset(warm_t, 0.0)
            cs_psum = pcs.tile([1, FP], f32)
            warm_ps = ps.tile([128, 256], f32, tag="ps", name="warm")

            # PE p-state pre-ramp: harmless matmuls on a zeroed tile keep
            # the tensor engine busy (and clocking up) while DMA streams.
            for i in range(N_WARM):
                nc.tensor.matmul(warm_ps, warm_t[:, :, 0:128], warm_t,
                                 start=True, stop=True, perf_mode=DR)

            # DMAs in consumption order (single hw queue, big descriptors)
            nc.sync.dma_start(out=we_t, in_=w_ev_d[:])
            cps = []
            for g in range(NGRP):
                ct = cp.tile([128, 2, NSLOT, 128], f8, tag="cp",
                             name=f"c{g}")
                cps.append(ct)
                nc.sync.dma_start(out=ct, in_=crops_d[g])
                if g == 0:
                    nc.sync.dma_start(out=wd_t, in_=w_od_d[:])

            for blk in range(NB):
                g, b = divmod(blk, 2)
                ee_t = ps.tile([128, 2 * FEE], f32, tag="ps", name=f"ee{blk}")
                eo_t = ps.tile([128, 2 * FEO], f32, tag="ps", name=f"eo{blk}")
                od_t = ps.tile([128, 2 * FOD], f32, tag="ps", name=f"od{blk}")
                # DoubleRow: lhsT [128, 2, 128] two k-tiles of crops,
                # rhs [128, 2, N<=256] same two k-tiles of W.
                for c in range(4):
                    nc.tensor.matmul(
                        ee_t, cps[g][:, b, 2 * c:2 * c + 2, :],
                        we_t[:, 2 * c:2 * c + 2, 0:2 * FEE],
                        start=(c == 0), stop=(c == 3), perf_mode=DR)
                for c in range(4):
                    nc.tensor.matmul(
                        eo_t, cps[g][:, b, 8 + 2 * c:8 + 2 * c + 2, :],
                        we_t[:, 2 * c:2 * c + 2, 2 * FEE:CEV],
                        start=(c == 0), stop=(c == 3), perf_mode=DR)
                for lo, hi in ((0, FOD), (FOD, COD)):
                    for c in range(8):
                        nc.tensor.matmul(
                            od_t[:, lo:hi],
                            cps[g][:, b, 16 + 2 * c:16 + 2 * c + 2, :],
                            wd_t[:, 2 * c:2 * c + 2, lo:hi],
                            start=(c == 0), stop=(c == 7), perf_mode=DR)

                # Post: PSD, row stats, normalized column sums.
                sq_ee = sqp.tile([128, 2 * FEE], f32, tag="sqe",
                                 name=f"se{blk}")
                sq_eo = sqp.tile([128, 2 * FEO], f32, tag="sqo",
                                 name=f"so{blk}")
                sq_od = sqp.tile([128, 2 * FOD], f32, tag="sqd",
                                 name=f"sd{blk}")
                a_ee = sm.tile([128, 1], f32, tag="ae", name=f"ae{blk}")
                a_eo = sm.tile([128, 1], f32, tag="ao", name=f"ao{blk}")
                a_od = sm.tile([128, 1], f32, tag="ad", name=f"ad{blk}")
                nc.scalar.activation(out=sq_ee, in_=ee_t, func=AF.Square,
                                     accum_out=a_ee)
                nc.scalar.activation(out=sq_eo, in_=eo_t, func=AF.Square,
                                     accum_out=a_eo)
                nc.scalar.activation(out=sq_od, in_=od_t, func=AF.Square,
                                     accum_out=a_od)
                p_t = ppool.tile([128, FP], f32r, tag="p", name=f"p{blk}")
                with nc.allow_low_precision(reason="fp32r is fp32-width"):
                    nc.vector.tensor_add(p_t[:, 0:FEE], sq_ee[:, 0:FEE],
                                         sq_ee[:, FEE:2 * FEE])
                    nc.vector.tensor_add(p_t[:, FEE:FEE + FEO],
                                         sq_eo[:, 0:FEO],
                                         sq_eo[:, FEO:2 * FEO])
                    nc.vector.tensor_add(p_t[:, FEE + FEO:F],
                                         sq_od[:, 0:FOD],
                                         sq_od[:, FOD:2 * FOD])
                    nc.vector.tensor_copy(p_t[:, F:FP], zero_col)
                rs = rq_t[:, 2 * blk:2 * blk + 1]
                tmp = sm.tile([128, 1], f32, tag="tmp", name=f"t{blk}")
                nc.vector.tensor_add(tmp, a_ee, a_eo)
                nc.vector.tensor_add(rs, tmp, a_od)
                psq = ppool.tile([128, F], f32, tag="psq", name=f"q{blk}")
                nc.scalar.activation(
                    out=psq, in_=p_t[:, 0:F], func=AF.Square,
                    accum_out=rq_t[:, 2 * blk + 1:2 * blk + 2])
                inv = sm.tile([128, 1], f32r, tag="inv", name=f"i{blk}")
                with nc.allow_low_precision(reason="fp32r is fp32-width"):
                    nc.vector.reciprocal(inv, rs)
                nc.tensor.matmul(cs_psum, inv, p_t,
                                 start=(blk == 0), stop=(blk == NB - 1))

            cs_sb = outp.tile([1, F], f32)
            nc.vector.tensor_copy(cs_sb, cs_psum[:, 0:F])
            nc.sync.dma_start(out=out_cs[:], in_=cs_sb)
            nc.sync.dma_start(out=out_rq[:], in_=rq_t)

    nc.compile()
    _NC = nc
    return nc


def _core_input(rows_ed):
    """rows_ed: (e, d) each [1024, 2048] f32 -> crops [4,128,2,32,128] fp8."""
    import ml_dtypes
    e, d = rows_ed
    # per crop 4096 samples in slot order: ee (8 slots), eo (8), d (16)
    q = np.concatenate([e[:, :QUART] + e[:, QUART:],
                        e[:, :QUART] - e[:, QUART:], d],
                       axis=1).astype(ml_dtypes.float8_e4m3)
    # [256g+128b+c, 128s+p] -> [g, p, b, s, c]
    arr = q.reshape(NGRP, 2, 128, NSLOT, 128).transpose(0, 4, 1, 3, 2)
    return {"crops": np.ascontiguousarray(arr)}


def _host_prepare(model_output, GT_sig, offsets_st, offsets_t):
    """Build per-core in_maps."""
    w_ev, w_od = _band_mats()
    from numpy.lib.stride_tricks import sliding_window_view
    in_maps = []
    mats = []   # 4 matrices' (e, d) row data [2048, 2048] each
    for b in range(B):
        offs = np.asarray(offsets_st[b], dtype=np.int64).reshape(-1)
        ch_idx = np.repeat(np.arange(C), K_CROPS)
        base = np.asarray(model_output[b], dtype=np.float32)
        win = sliding_window_view(base, L, axis=-1)  # [C, T-L+1, L]
        cr = win[ch_idx, offs]                       # [2048, L]
        mats.append((cr[:, :HALF] + cr[:, HALF:],
                     cr[:, :HALF] - cr[:, HALF:]))
    for b in range(B):
        offs = np.asarray(offsets_t[b], dtype=np.int64).reshape(-1)
        win = sliding_window_view(
            np.asarray(GT_sig[b], dtype=np.float32), L)
        cr = win[offs]
        mats.append((cr[:, :HALF] + cr[:, HALF:],
                     cr[:, :HALF] - cr[:, HALF:]))
    for m in range(4):
        e, d = mats[m]
        for h in range(2):
            sl = slice(h * ROWS_PER_CORE, (h + 1) * ROWS_PER_CORE)
            im = {"w_ev": w_ev, "w_od": w_od}
            im.update(_core_input((e[sl], d[sl])))
            in_maps.append(im)
    return in_maps


def _combine(results, label_flag):
    """results: list of 8 dicts with out_cs [1,F], out_rq [128,2*NB]."""
    cs = np.zeros((4, F), dtype=np.float64)
    ssq = np.zeros(4, dtype=np.float64)
    for m in range(4):
        for h in range(2):
            r = results[2 * m + h]
            cs[m] += np.asarray(r["out_cs"], dtype=np.float64)[0]
            rq = np.asarray(r["out_rq"], dtype=np.float64)
            rs = rq[:, 0::2]
            q = rq[:, 1::2]
            ssq[m] += float(np.sum(q / (rs * rs)))

    N = float(N_ROWS)

    def cmp_excl(a):
        return (2.0 * N * ssq[a] - 2.0 * np.dot(cs[a], cs[a])) / F / (N * (N - 1.0))

    def cmp_full(a, b):
        return (N * ssq[a] + N * ssq[b] - 2.0 * np.dot(cs[a], cs[b])) / F / (N * N)

    lf = np.asarray(label_flag, dtype=np.float64).reshape(-1)
    lf_sum = lf[0] + lf[1]
    denom = 1.0 if lf_sum == 0 else lf_sum
    pos_loss = (cmp_excl(0) + cmp_excl(1)) / 2.0
    neg_loss = -cmp_full(0, 1)
    pos_GT = (lf[0] * cmp_full(0, 2) + lf[1] * cmp_full(1, 3)) / denom
    neg_GT = -(lf[0] * cmp_full(1, 2) + lf[1] * cmp_full(0, 3)) / denom
    if lf_sum == 0:
        pos_GT = 0.0
        neg_GT = 0.0
    loss = pos_loss + neg_loss + pos_GT + neg_GT
    return (np.float32(loss), np.float32(pos_loss), np.float32(neg_loss),
            np.float32(pos_GT), np.float32(neg_GT))


def run(inputs, trace=False):
    """Returns (outputs_tuple, BassKernelResults)."""
    from concourse import bass_utils
    nc = _build_module()
    in_maps = _host_prepare(
        inputs["model_output"], inputs["GT_sig"],
        inputs["offsets_st"], inputs["offsets_t"])
    res = bass_utils.run_bass_kernel_spmd(
        nc, in_maps, core_ids=list(range(N_CORES)), trace=trace)
    outs = _combine(res.results, inputs["label_flag"])
    return outs, res


def kernel(**inputs):
    outs, _ = run(inputs)
    return outs


# revision 13
# speedup vs baseline: 2.5799x; 1.2017x over previous
"""Trainium2 Bass kernel for nn_ContrastLoss (contrastive PSD loss).

Math notes (validated against the jax reference and a numpy emulator):
  * The band (rfft bins 92..568 of a 4096-point DFT) excludes DC, so the
    mean subtraction in the reference is a no-op for the band PSD.
  * diag(D) == 0 for the pairwise-MSE matrix, and every _compare() term
    reduces to rank-1 statistics of the normalized PSD matrices:
        sum_ij D_ij * F = M*SSQ_a + N*SSQ_b - 2 * cs_a . cs_b
    with SSQ = sum of squared entries and cs = column sums.  So the NxN
    Gram matrix is never materialized; the device only produces per-core
    column sums and per-row (sum, sum-of-squares) statistics.
  * Radix-2 DIF recursion on the crop halves splits the band bins into
    classes by k mod 32 with REAL sub-signals (x0+-x1 folds, applied
    where the class phases allow it):
        od    k odd       : d     [2048]   238 bins
        eo    k = 2 mod 4 : eo    [1024]   119 bins
        eeo   k = 4 mod 8 : eeo   [512]     60 bins
        eeeo  k = 8 mod 16: eeeo  [256]     30 bins
        eeeee k = 0 mod 32: eeeee [128]     15 bins
        eeeeo k =16 mod 32: eeeeo [128]     15 bins
  * Universal symmetric fold: for each class, cos(theta(n-m,k)) =
    -cos(theta(m,k)) and sin(theta(n-m,k)) = +sin(theta(m,k)), so
        Re X = [s_0, s_m - s_{n-m}] . cos-matrix     (n/2 contraction)
        Im X = [s_{n/2}, s_m + s_{n-m}] . -sin-matrix (n/2 contraction)
    (edge samples ride in the free m=0 slot; cos theta(n/2,k) = 0 and
    sin theta(0,k) = 0 for these classes).  This halves the matmul MACs
    again: 2620 PE cycles per 128-crop block, DFT matrices 655 KB.
  * All matmul operands are fp8-e4m3; >=256-contractions use DoubleRow
    perf mode (two 128-deep k-tiles per instruction).  End-to-end e4m3
    error: ~4e-6 on the loss terms, ~3e-3 on the cancellation-dominated
    total loss, far under the 2e-2 gate.

Device schedule per core (1024 crops of the 8192 total):
  Big-line DMAs (one descriptor per partition): W blob [128,2,2560]
  (5 KB lines) then 8 crop blocks [128,32,128] (4 KB lines).  Dummy fp8
  matmuls pre-ramp the PE p-state while DMA streams.  Per block: 20
  matmuls -> 2 PSUM tiles, ACT Square -> sq, DVE adds -> band PSD
  p [128,477], DVE row-reduce -> rowsum, ACT Square+accum -> rowsumsq,
  PE colsum matmul with lhsT = 1/rowsum -> cs [1,477].  Host combines
  the 8 cores' (cs, rowstats) in float64.
"""

import numpy as np

# Problem constants (hardcoded; kernel.py must be self-contained)
B, C, T = 2, 64, 32768
L = 4096
K_CROPS = 32
HALF = L // 2                  # 2048
N_ROWS = C * K_CROPS           # 2048 rows per PSD matrix
N_CORES = 8
ROWS_PER_CORE = N_ROWS * 4 // N_CORES   # 1024
NB = ROWS_PER_CORE // 128      # 8 row blocks per core
NSLOT = 32                     # 128-sample slots per crop
N_WARM = 22                    # PE p-state pre-ramp matmuls

# Band-bin classes: (kset, signal length n, folded?)
K_EEEEE = np.arange(96, 545, 32)    # 15 bins, k=0 mod 32
K_EEEEO = np.arange(112, 561, 32)   # 15 bins, k=16 mod 32
K_EEEO = np.arange(104, 569, 16)    # 30 bins, k=8 mod 16
K_EEO = np.arange(92, 565, 8)       # 60 bins, k=4 mod 8
K_EO = np.arange(94, 567, 4)        # 119 bins, k=2 mod 4
K_OD = np.arange(93, 568, 2)        # 238 bins, k odd
F = 477
FP = F + 1                     # pad col so fp32r matmul free dim is even
WX = 2560                      # W blob free columns (per k-tile slot)

_NC = None
_W_CACHE = None


def _w_blob():
    """fp8-e4m3 DFT matrices packed as [128, 2, WX].

    dim1 indexes the two k-tiles of a DoubleRow pair (or t=0/1 reuse for
    the four plain 128-contraction classes).  Layout in the X dim:
      [0:30)    t0 eeeee [cos|-sin], t1 eeeeo
      [30:60)   t0 eeeo-cos,         t1 eeeo-sin
      [60:120)  eeo-cos   [120:180) eeo-sin      (1 pair)
      [180:418) eo-cos x2 [418:656) eo-sin x2    (119 cols per pair)
      [656:1608) od-cos x4 [1608:2560) od-sin x4 (238 cols per pair)
    """
    global _W_CACHE
    if _W_CACHE is not None:
        return _W_CACHE
    import ml_dtypes

    def ang(ms, ks):
        return 2.0 * np.pi * (ms[:, None].astype(np.float64) *
                              ks[None, :]) / float(L)

    blob = np.zeros((128, 2, WX), dtype=np.float32)

    def put_plain(t, x0, ks):
        a = ang(np.arange(128), ks)
        blob[:, t, x0:x0 + 2 * len(ks)] = np.concatenate(
            [np.cos(a), -np.sin(a)], axis=1)

    def put_fold(x0c, x0s, ks, n):
        h = n // 2
        mc = np.concatenate([[0], np.arange(1, h)])
        ms = np.concatenate([[h], np.arange(1, h)])
        wc = np.cos(ang(mc, ks))           # [h, nb]
        ws = -np.sin(ang(ms, ks))
        npair = h // 256
        nb = len(ks)
        for w, x0 in ((wc, x0c), (ws, x0s)):
            r = w.reshape(npair, 2, 128, nb).transpose(2, 1, 0, 3)
            blob[:, :, x0:x0 + npair * nb] = r.reshape(128, 2, npair * nb)

    put_plain(0, 0, K_EEEEE)
    put_plain(1, 0, K_EEEEO)
    # eeeo: folded to 128-contraction -> plain slots t0/t1 at [30:60)
    h = 128
    mc = np.concatenate([[0], np.arange(1, h)])
    ms = np.concatenate([[h], np.arange(1, h)])
    blob[:, 0, 30:60] = np.cos(ang(mc, K_EEEO))
    blob[:, 1, 30:60] = -np.sin(ang(ms, K_EEEO))
    put_fold(60, 120, K_EEO, 512)
    put_fold(180, 418, K_EO, 1024)
    put_fold(656, 1608, K_OD, 2048)

    _W_CACHE = np.ascontiguousarray(blob.astype(ml_dtypes.float8_e4m3))
    return _W_CACHE


def _build_module():
    global _NC
    if _NC is not None:
        return _NC
    import concourse.bacc as bacc
    import concourse.bass as bass
    import concourse.tile as tile
    from concourse import mybir

    f32 = mybir.dt.float32
    f32r = mybir.dt.float32r
    f8 = mybir.dt.float8e4
    AF = mybir.ActivationFunctionType
    DR = mybir.MatmulPerfMode.DoubleRow

    nc = bacc.Bacc("TRN2", target_bir_lowering=False, debug=False,
                   num_devices=N_CORES)

    # crops: [blk, partition, slot, crop] fp8, 4 KB lines
    crops_d = nc.dram_tensor("crops", [NB, 128, NSLOT, 128], f8,
                             kind="ExternalInput")
    w_d = nc.dram_tensor("w", [128, 2, WX], f8, kind="ExternalInput")
    out_cs = nc.dram_tensor("out_cs", [1, F], f32, kind="ExternalOutput")
    out_rq = nc.dram_tensor("out_rq", [128, 2 * NB], f32,
                            kind="ExternalOutput")

    with tile.TileContext(nc) as tc:
        with (
            tc.tile_pool(name="wp", bufs=1) as wp,
            tc.tile_pool(name="cp", bufs=NB) as cp,
            tc.tile_pool(name="sq", bufs=4) as sqp,
            tc.tile_pool(name="pp", bufs=3) as ppool,
            tc.tile_pool(name="sm", bufs=8) as sm,
            tc.tile_pool(name="outp", bufs=1) as outp,
            tc.tile_pool(name="ps", bufs=6, space=bass.MemorySpace.PSUM) as ps,
            tc.tile_pool(name="psw", bufs=1, space=bass.MemorySpace.PSUM) as psw,
            tc.tile_pool(name="pcs", bufs=1, space=bass.MemorySpace.PSUM) as pcs,
        ):
            wb = wp.tile([128, 2, WX], f8)
            rq_t = outp.tile([128, 2 * NB], f32)
            zero_col = outp.tile([128, 1], f32)
            warm_t = outp.tile([128, 2, 256], f8)
            nc.vector.memset(zero_col, 0.0)
            nc.vector.memset(warm_t, 0.0)
            cs_psum = pcs.tile([1, FP], f32)
            warm_ps = psw.tile([128, 256], f32)

            # PE p-state pre-ramp while DMA streams.
            for i in range(N_WARM):
                nc.tensor.matmul(warm_ps, warm_t[:, :, 0:128], warm_t,
                                 start=True, stop=True, perf_mode=DR)

            # DMAs in consumption order (big descriptors, one hw queue)
            nc.sync.dma_start(out=wb, in_=w_d[:])
            cpb = []
            for blk in range(NB):
                ct = cp.tile([128, NSLOT, 128], f8, tag="cp", name=f"c{blk}")
                cpb.append(ct)
                nc.sync.dma_start(out=ct, in_=crops_d[blk])

            for blk in range(NB):
                cb = cpb[blk]
                ev_t = ps.tile([128, 478], f32, tag="ps", name=f"ev{blk}")
                od_t = ps.tile([128, 476], f32, tag="ps", name=f"od{blk}")
                # plain 128-contraction classes (single k-tile)
                for x0, slot, t, wx0 in ((0, 0, 0, 0), (30, 1, 1, 0),
                                         (60, 2, 0, 30), (90, 3, 1, 30)):
                    nc.tensor.matmul(ev_t[:, x0:x0 + 30], cb[:, slot, :],
                                     wb[:, t, wx0:wx0 + 30],
                                     start=True, stop=True)
                # DoubleRow folded classes: (psum tile, psum x0, slot0,
                #   blob x0, cols, pairs)
                for p_t_, px0, s0, wx0, nb, npair in (
                        (ev_t, 120, 4, 60, 60, 1),
                        (ev_t, 180, 6, 120, 60, 1),
                        (ev_t, 240, 8, 180, 119, 2),
                        (ev_t, 359, 12, 418, 119, 2),
                        (od_t, 0, 16, 656, 238, 4),
                        (od_t, 238, 24, 1608, 238, 4)):
                    for c in range(npair):
                        nc.tensor.matmul(
                            p_t_[:, px0:px0 + nb],
                            cb[:, s0 + 2 * c:s0 + 2 * c + 2, :],
                            wb[:, :, wx0 + nb * c:wx0 + nb * (c + 1)],
                            start=(c == 0), stop=(c == npair - 1),
                            perf_mode=DR)

                # Post: PSD, row stats, normalized column sums.
                sq_ev = sqp.tile([128, 478], f32, tag="sqe", name=f"se{blk}")
                sq_od = sqp.tile([128, 476], f32, tag="sqd", name=f"sd{blk}")
                nc.scalar.activation(out=sq_ev, in_=ev_t, func=AF.Square)
                nc.scalar.activation(out=sq_od, in_=od_t, func=AF.Square)
                p_t = ppool.tile([128, FP], f32r, tag="p", name=f"p{blk}")
                with nc.allow_low_precision(reason="fp32r is fp32-width"):
                    for dst, a, b_ in ((0, 0, 15), (15, 30, 45),
                                       (30, 60, 90), (60, 120, 180),
                                       (120, 240, 359)):
                        n = b_ - a
                        nc.vector.tensor_add(p_t[:, dst:dst + n],
                                             sq_ev[:, a:a + n],
                                             sq_ev[:, b_:b_ + n])
                    nc.vector.tensor_add(p_t[:, 239:477], sq_od[:, 0:238],
                                         sq_od[:, 238:476])
                    nc.vector.tensor_copy(p_t[:, F:FP], zero_col)
                rs = rq_t[:, 2 * blk:2 * blk + 1]
                nc.vector.tensor_reduce(rs, p_t[:, 0:F],
                                        axis=mybir.AxisListType.X,
                                        op=mybir.AluOpType.add)
                psq = ppool.tile([128, F], f32, tag="psq", name=f"q{blk}")
                nc.scalar.activation(
                    out=psq, in_=p_t[:, 0:F], func=AF.Square,
                    accum_out=rq_t[:, 2 * blk + 1:2 * blk + 2])
                inv = sm.tile([128, 1], f32r, tag="inv", name=f"i{blk}")
                with nc.allow_low_precision(reason="fp32r is fp32-width"):
                    nc.vector.reciprocal(inv, rs)
                nc.tensor.matmul(cs_psum, inv, p_t,
                                 start=(blk == 0), stop=(blk == NB - 1))

            cs_sb = outp.tile([1, F], f32)
            nc.vector.tensor_copy(cs_sb, cs_psum[:, 0:F])
            nc.sync.dma_start(out=out_cs[:], in_=cs_sb)
            nc.sync.dma_start(out=out_rq[:], in_=rq_t)

    nc.compile()
    _NC = nc
    return nc


def _fold_cs(sig):
    """sig [R, n] -> (cos fold [R, n/2], sin fold [R, n/2])."""
    n = sig.shape[1]
    h = n // 2
    c = np.empty((sig.shape[0], h), dtype=np.float32)
    s = np.empty_like(c)
    c[:, 0] = sig[:, 0]
    s[:, 0] = sig[:, h]
    c[:, 1:] = sig[:, 1:h] - sig[:, :h:-1]
    s[:, 1:] = sig[:, 1:h] + sig[:, :h:-1]
    return c, s


def _core_input(rows_ed):
    """rows_ed: (e, d) each [1024, 2048] f32 -> crops [8,128,32,128] fp8."""
    import ml_dtypes
    e, d = rows_ed
    ee = e[:, :1024] + e[:, 1024:]
    eo = e[:, :1024] - e[:, 1024:]
    eee = ee[:, :512] + ee[:, 512:]
    eeo = ee[:, :512] - ee[:, 512:]
    eeee = eee[:, :256] + eee[:, 256:]
    eeeo = eee[:, :256] - eee[:, 256:]
    eeeee = eeee[:, :128] + eeee[:, 128:]
    eeeeo = eeee[:, :128] - eeee[:, 128:]
    parts = [eeeee, eeeeo]
    for sig in (eeeo, eeo, eo, d):
        parts.extend(_fold_cs(sig))
    q = np.concatenate(parts, axis=1).astype(ml_dtypes.float8_e4m3)
    # [128b+cr, 128s+p] -> [b, p, s, cr]
    arr = q.reshape(NB, 128, NSLOT, 128).transpose(0, 3, 2, 1)
    return {"crops": np.ascontiguousarray(arr)}


def _host_prepare(model_output, GT_sig, offsets_st, offsets_t):
    """Build per-core in_maps."""
    w_blob = _w_blob()
    from numpy.lib.stride_tricks import sliding_window_view
    in_maps = []
    mats = []   # 4 matrices' (e, d) row data [2048, 2048] each
    for b in range(B):
        offs = np.asarray(offsets_st[b], dtype=np.int64).reshape(-1)
        ch_idx = np.repeat(np.arange(C), K_CROPS)
        base = np.asarray(model_output[b], dtype=np.float32)
        win = sliding_window_view(base, L, axis=-1)  # [C, T-L+1, L]
        cr = win[ch_idx, offs]                       # [2048, L]
        mats.append((cr[:, :HALF] + cr[:, HALF:],
                     cr[:, :HALF] - cr[:, HALF:]))
    for b in range(B):
        offs = np.asarray(offsets_t[b], dtype=np.int64).reshape(-1)
        win = sliding_window_view(
            np.asarray(GT_sig[b], dtype=np.float32), L)
        cr = win[offs]
        mats.append((cr[:, :HALF] + cr[:, HALF:],
                     cr[:, :HALF] - cr[:, HALF:]))
    for m in range(4):
        e, d = mats[m]
        for h in range(2):
            sl = slice(h * ROWS_PER_CORE, (h + 1) * ROWS_PER_CORE)
            im = {"w": w_blob}
            im.update(_core_input((e[sl], d[sl])))
            in_maps.append(im)
    return in_maps


def _combine(results, label_flag):
    """results: list of 8 dicts with out_cs [1,F], out_rq [128,2*NB]."""
    cs = np.zeros((4, F), dtype=np.float64)
    ssq = np.zeros(4, dtype=np.float64)
    for m in range(4):
        for h in range(2):
            r = results[2 * m + h]
            cs[m] += np.asarray(r["out_cs"], dtype=np.float64)[0]
            rq = np.asarray(r["out_rq"], dtype=np.float64)
            rs = rq[:, 0::2]
            q = rq[:, 1::2]
            ssq[m] += float(np.sum(q / (rs * rs)))

    N = float(N_ROWS)

    def cmp_excl(a):
        return (2.0 * N * ssq[a] - 2.0 * np.dot(cs[a], cs[a])) / F / (N * (N - 1.0))

    def cmp_full(a, b):
        return (N * ssq[a] + N * ssq[b] - 2.0 * np.dot(cs[a], cs[b])) / F / (N * N)

    lf = np.asarray(label_flag, dtype=np.float64).reshape(-1)
    lf_sum = lf[0] + lf[1]
    denom = 1.0 if lf_sum == 0 else lf_sum
    pos_loss = (cmp_excl(0) + cmp_excl(1)) / 2.0
    neg_loss = -cmp_full(0, 1)
    pos_GT = (lf[0] * cmp_full(0, 2) + lf[1] * cmp_full(1, 3)) / denom
    neg_GT = -(lf[0] * cmp_full(1, 2) + lf[1] * cmp_full(0, 3)) / denom
    if lf_sum == 0:
        pos_GT = 0.0
        neg_GT = 0.0
    loss = pos_loss + neg_loss + pos_GT + neg_GT
    return (np.float32(loss), np.float32(pos_loss), np.float32(neg_loss),
            np.float32(pos_GT), np.float32(neg_GT))


def run(inputs, trace=False):
    """Returns (outputs_tuple, BassKernelResults)."""
    from concourse import bass_utils
    nc = _build_module()
    in_maps = _host_prepare(
        inputs["model_output"], inputs["GT_sig"],
        inputs["offsets_st"], inputs["offsets_t"])
    res = bass_utils.run_bass_kernel_spmd(
        nc, in_maps, core_ids=list(range(N_CORES)), trace=trace)
    outs = _combine(res.results, inputs["label_flag"])
    return outs, res


def kernel(**inputs):
    outs, _ = run(inputs)
    return outs
